# revision 43
# baseline (speedup 1.0000x reference)
"""Butterfly (nn_Butterfly) kernel for 8 Trainium2 NeuronCores — v5.

Math: stages 0-4 of the 10-stage butterfly mix features within contiguous
32-blocks (factor A), stages 5-9 mix features with equal (p mod 32)
(factor B); out = x A^T B^T + bias. Each core runs its 4096-sample batch
shard through three PE passes per 1024-sample chunk:

  phase A   8 matmuls per 128-sample slab with the *data* slab as the
            stationary operand and the A-factor tile as the moving one, so
            the PSUM result comes out sample-major (the first transpose of
            the feature exchange is fused into the matmul for free)
  T2        64 [128,128] PE transposes flip the intermediate back to
            feature-major, grouped so each transposed block is exactly one
            phase-B contraction tile (the full pi exchange = A-fusion + T2)
  phase B   16 matmuls per chunk; bias is added on the host during unpack

The PE stream is software-pipelined two chunks deep
(T2(j-1)/B(j-2) interleaved at slab granularity, then A(j)) so the PE
never stalls on PSUM evacuations; evacuations are spread across
DVE/ACT/Pool so no engine exceeds the DMA period. The DMA device (serial
~360GB/s in the cost model) carries only the 8MB input + 8MB output +
0.5MB weights per core; the feature exchange never touches it.
"""

import os
import numpy as np
import ml_dtypes

import concourse.bass as bass
import concourse.bacc as bacc
import concourse.mybir as mybir
import concourse.tile as tile
from concourse.bass_utils import run_bass_kernel_spmd

N_FEAT = 1024
M_STAGES = 10
N_CORES = 8

BF16 = ml_dtypes.bfloat16

LAST_EXEC_NS = None  # set when BASS_KERNEL_TRACE=1
CHUNK = 512

_CACHE = {}


def _apply_stages(x, twiddle, blocks):
    """Apply butterfly stages `blocks` to x [b, 1024] (mirrors reference)."""
    n = N_FEAT
    for m in blocks:
        s = 1 << m
        t = twiddle[0, m].reshape(n // (2 * s), s, 2, 2)
        o = x.reshape(-1, n // (2 * s), 2, s)
        x = np.einsum("gsij,bgjs->bgis", t, o).reshape(-1, n)
    return x


def _phase_mats(twiddle):
    """Device weight layouts.

    wa [128, 8*128]: tile c holds the A-factor for input features
      [128c, 128c+128); column f = 16*r_hi + 4*a_lo + r_lo is output
      feature 32*(4c + a_lo) + 4*r_hi + r_lo. Used as the MOVING operand
      (data slab stationary), so phase-A PSUM is sample-major.
    wb [128, 8*128]: tile r_hi contracts the 128 intermediate features
      q = 4*a + r_lo (feature 32a + 4 r_hi + r_lo) into outputs
      g = 4*a' + r_lo (feature 32a' + 4 r_hi + r_lo).
    """
    tw = twiddle.astype(np.float64)
    eye = np.eye(N_FEAT)
    a_full = _apply_stages(eye, tw, range(5)).T      # [out_feat, in_feat]
    b_full = _apply_stages(eye, tw, range(5, 10)).T
    f = np.arange(128)
    r_hi, a_lo, r_lo = f >> 4, (f >> 2) & 3, f & 3
    k = np.arange(128)
    wa = np.zeros((128, 8 * 128))
    wb = np.zeros((128, 8 * 128))
    for c in range(8):
        p_out = 32 * (4 * c + a_lo) + 4 * r_hi + r_lo
        p_in = 128 * c + k
        wa[:, c * 128:(c + 1) * 128] = a_full[np.ix_(p_out, p_in)].T
    g = np.arange(128)
    for rh in range(8):
        feat = 32 * (g >> 2) + 4 * rh + (g & 3)
        wb[:, rh * 128:(rh + 1) * 128] = b_full[np.ix_(feat, feat)].T
    return wa.astype(BF16), wb.astype(BF16)


def _pack_xt(shard_bf, chunk):
    """[bpc, 1024] bf16 -> [128, 8*bpc] slab-major device layout:
    xt[k, j*8*chunk + s*1024 + c*128 + n0] = x[j*chunk + s*128 + n0, c*128 + k]."""
    bpc = shard_bf.shape[0]
    nch = bpc // chunk
    ns = chunk // 128
    a = shard_bf.reshape(nch, ns, 128, 8, 128)      # [j, s, n0, c, k]
    return np.ascontiguousarray(
        a.transpose(4, 0, 1, 3, 2).reshape(128, 8 * bpc)
    )


def _unpack_out(raw, chunk, bias):
    """device out [128, 8*bpc] -> [bpc, 1024] f32 (+ bias on host).
    raw[g, j*8*chunk + rh*chunk + n] = out[j*chunk + n, 32*(g>>2) + 4*rh + (g&3)].
    """
    bpc = raw.shape[1] // 8
    nch = bpc // chunk
    raw = np.asarray(raw).astype(np.float32)
    a = raw.reshape(32, 4, nch, 8, chunk)           # [a', r_lo, j, rh, n]
    out = np.ascontiguousarray(a.transpose(2, 4, 0, 3, 1).reshape(bpc, N_FEAT))
    out += bias[None, :]
    return out


def _build_program_v5(bpc, chunk=1024, eva=None, evb=None, xbar_every=0):
    """Three-PE-pass butterfly for one core's shard (see module docstring)."""
    from concourse.bass import AP as _AP
    from concourse.bass import _add_dep_helper

    assert chunk % 512 == 0 and bpc % chunk == 0
    nch = bpc // chunk
    F = 8 * chunk                      # columns per chunk in xin/M/saq/ot
    NS = chunk // 128                  # sample slabs per chunk

    nc = bacc.Bacc("TRN2", debug=False)
    xt_d = nc.dram_tensor("xt", [128, 8 * bpc], mybir.dt.bfloat16, kind="ExternalInput").ap()
    wa_d = nc.dram_tensor("wa", [128, 8 * 128], mybir.dt.bfloat16, kind="ExternalInput").ap()
    wb_d = nc.dram_tensor("wb", [128, 8 * 128], mybir.dt.bfloat16, kind="ExternalInput").ap()
    id_d = nc.dram_tensor("idm", [128, 128], mybir.dt.bfloat16, kind="ExternalInput").ap()
    out_d = nc.dram_tensor("outqT", [128, 8 * bpc], mybir.dt.bfloat16, kind="ExternalOutput").ap()

    with tile.TileContext(nc) as tc:
        with (
            tc.tile_pool(name="wpool", bufs=1) as w_pool,
            tc.tile_pool(name="xin", bufs=4) as xin_pool,
            tc.tile_pool(name="mtile", bufs=2) as m_pool,
            tc.tile_pool(name="saq", bufs=2) as saq_pool,
            tc.tile_pool(name="otile", bufs=2) as out_pool,
            tc.tile_pool(name="psa", bufs=3, space="PSUM") as psa_pool,
            tc.tile_pool(name="pst", bufs=2, space="PSUM") as pst_pool,
            tc.tile_pool(name="psb", bufs=3, space="PSUM") as psb_pool,
        ):
            wa_sb = w_pool.tile([128, 8 * 128], mybir.dt.bfloat16, name="wa_sb")
            wb_sb = w_pool.tile([128, 8 * 128], mybir.dt.bfloat16, name="wb_sb")
            id_sb = w_pool.tile([128, 128], mybir.dt.bfloat16, name="id_sb")

            # SP issues almost every DMA. The first two transfers (wa + the
            # first input quarter) go out on scalar's HWDGE concurrently so
            # the ~1.3us issue latency of the two queues overlaps.
            nc.scalar.dma_start(wa_sb[:], wa_d[:])

            xins = []

            def load_chunk(j, pieces=2, first_eng=None):
                xin = xin_pool.tile([128, F], mybir.dt.bfloat16, name="xin")
                w = F // pieces
                for h in range(pieces):
                    eng = first_eng if (h == 0 and first_eng is not None) else nc.sync
                    eng.dma_start(
                        xin[:, h * w:(h + 1) * w],
                        xt_d[:, j * F + h * w: j * F + (h + 1) * w],
                    )
                xins.append(xin)

            load_chunk(0, pieces=4)
            nc.sync.dma_start(wb_sb[:], wb_d[:])
            nc.sync.dma_start(id_sb[:], id_d[:])
            if nch > 1:
                load_chunk(1)
            if nch > 2:
                load_chunk(2)

            # engine split patterns. GPSIMD cannot access PSUM on TRN2
            # (BIR verifier), so every evacuation is on DVE or ACT:
            # per chunk DVE carries the 8 evacT2 (2x bf16) + 11 f32 halves,
            # ACT the other 21 halves (~12us each, the pipeline period).
            EMAP = {"S": nc.scalar, "V": nc.vector}
            EVA = [EMAP[ch] for ch in (eva or "VSSSVSSSVSSSVSSS")]     # evacA
            EVB = [EMAP[ch] for ch in (evb or "SVSVSVSSVVSSVSSV")]     # evacB
            EVB_TAIL = [EMAP[ch] for ch in "VSVSVSVSVSVSVSVS"]
            # tail chunks: ACT's backlog gates the drain; shift the last
            # chunks' evacA to DVE, which idles there.
            EVA_TAIL = [EMAP[ch] for ch in "VSVSVSVSVSVSVSVS"]

            def copy_on(eng, dst, src):
                if eng is nc.scalar:
                    return eng.activation(dst, src, mybir.ActivationFunctionType.Identity)
                return eng.tensor_copy(dst, src)

            # per-chunk state
            st = {}
            for j in range(nch):
                st[j] = {"evacA": {}, "evacT2": {}, "M": None, "saq": None,
                         "ot": None}

            def emit_A_slab(j, s):
                xin = xins[j]
                if st[j]["M"] is None:
                    st[j]["M"] = m_pool.tile([128, F], mybir.dt.bfloat16, name="mtile")
                    st[j]["evacA"] = {ss: [] for ss in range(NS)}
                m_h = st[j]["M"][:].tensor
                for h in range(2):
                    psa = psa_pool.tile([128, 512], mybir.dt.float32, name="psa")
                    for cc in range(4):
                        c = 4 * h + cc
                        nc.tensor.matmul(
                            psa[:, cc * 128:(cc + 1) * 128],
                            xin[:, s * 1024 + c * 128: s * 1024 + (c + 1) * 128],
                            wa_sb[:, c * 128:(c + 1) * 128],
                            start=True, stop=True,
                        )
                    # psa col cc*128 + (16 r_hi + z) -> M col
                    #   s*1024 + r_hi*128 + (4h+cc)*16 + z
                    dst = _AP(
                        m_h, s * 1024 + 64 * h,
                        [[F, 128], [16, 4], [128, 8], [1, 16]],
                    )
                    pat = EVA_TAIL if j >= nch - 1 else EVA
                    cp = copy_on(pat[(s * 2 + h) % len(pat)], dst, psa[:])
                    st[j]["evacA"][s].append(cp)

            def alloc_saq(j):
                saq = saq_pool.tile([128, F], mybir.dt.bfloat16, name="saq")
                st[j]["saq"] = saq
                st[j]["ot"] = out_pool.tile([128, F], mybir.dt.bfloat16, name="ot")

            def emit_T2_slab(j, s, use_xbar=False):
                M = st[j]["M"]
                saq = st[j]["saq"]
                if use_xbar:
                    # X-bar DMA transpose straight into saq; spends DMA-device
                    # slack to relieve the saturated DVE/PE.
                    last = None
                    for rh in range(8):
                        tp = nc.sync.dma_start_transpose(
                            saq[:, rh * chunk + s * 128: rh * chunk + (s + 1) * 128],
                            M[:, s * 1024 + rh * 128: s * 1024 + (rh + 1) * 128],
                        )
                        for cp in st[j]["evacA"][s]:
                            _add_dep_helper(tp.ins, cp.ins, sync=True,
                                            reason="xbar reads evacA raw-AP writes")
                        last = tp
                    st[j]["evacT2"][s] = last
                    return
                pst = pst_pool.tile([128, 1024], mybir.dt.bfloat16, name="pst")
                for rh in range(8):
                    tp = nc.tensor.transpose(
                        pst[:, rh * 128:(rh + 1) * 128],
                        M[:, s * 1024 + rh * 128: s * 1024 + (rh + 1) * 128],
                        id_sb[:],
                    )
                    for cp in st[j]["evacA"][s]:
                        _add_dep_helper(tp.ins, cp.ins, sync=True,
                                        reason="T2 reads evacA raw-AP writes")
                # pst col rh*128 + n0 -> saq col rh*chunk + s*128 + n0
                dst = _AP(saq[:].tensor, s * 128, [[F, 128], [chunk, 8], [1, 128]])
                st[j]["evacT2"][s] = nc.vector.tensor_copy(dst, pst[:])

            def emit_B_tile(j, rh):
                saq = st[j]["saq"]
                ot = st[j]["ot"]
                for t in range(chunk // 512):
                    psb = psb_pool.tile([128, 512], mybir.dt.float32, name="psb")
                    mm = nc.tensor.matmul(
                        psb[:],
                        wb_sb[:, rh * 128:(rh + 1) * 128],
                        saq[:, rh * chunk + t * 512: rh * chunk + (t + 1) * 512],
                        start=True, stop=True,
                    )
                    for s in range(4 * t, 4 * t + 4):
                        _add_dep_helper(mm.ins, st[j]["evacT2"][s].ins, sync=True,
                                        reason="B reads evacT2 raw-AP writes")
                    dst = ot[:, rh * chunk + t * 512: rh * chunk + (t + 1) * 512]
                    copy_on((EVB_TAIL if j >= nch - 1 else EVB)[(rh * (chunk // 512) + t) % len(EVB)], dst, psb[:])

            def emit_out(j, q, pieces=4):
                w = F // pieces
                nc.sync.dma_start(
                    out_d[:, j * F + q * w: j * F + (q + 1) * w],
                    st[j]["ot"][:, q * w:(q + 1) * w],
                )

            # software pipeline, two chunks deep on the PE stream, fully
            # interleaved at slab granularity so every PSUM slot has ~3.8us
            # of PE work between produce and reuse:
            #   step j, slab s: T2(j-1, s); B(j-2, rh=s) both halves; A(j, s)
            # Input loads go at step START (xin bufs=4 makes them dep-free
            # immediately) so the serial DMA device never sits behind an
            # output that is still waiting on its evacuation.
            RPB = 8 // NS
            for j in range(nch + 2):
                if j < nch and j + 3 < nch:
                    load_chunk(j + 3)
                if 1 <= j <= nch:
                    alloc_saq(j - 1)
                for s in range(NS):
                    if 1 <= j <= nch:
                        emit_T2_slab(j - 1, s)
                    if 2 <= j:
                        for rr in range(RPB):
                            emit_B_tile(j - 2, s * RPB + rr)
                        emit_out(j - 2, s, pieces=NS)
                    if j < nch:
                        emit_A_slab(j, s)

    nc.compile()
    return nc


def kernel(x, twiddle, bias):
    global LAST_EXEC_NS
    batch = x.shape[0]
    assert batch % N_CORES == 0
    bpc = batch // N_CORES
    chunk = CHUNK

    # ---- host prep ----
    wa, wb = _phase_mats(np.asarray(twiddle, dtype=np.float32))
    bias_f = np.asarray(bias, dtype=np.float32)
    idm = np.eye(128, dtype=BF16)
    x_bf = np.asarray(x).astype(BF16)
    shards = [
        _pack_xt(x_bf[k * bpc:(k + 1) * bpc, :], chunk)
        for k in range(N_CORES)
    ]

    key = ("v5", bpc, chunk)
    if key not in _CACHE:
        _CACHE[key] = _build_program_v5(bpc, chunk=chunk)
    nc = _CACHE[key]

    in_maps = [
        {"xt": shards[k], "wa": wa, "wb": wb, "idm": idm}
        for k in range(N_CORES)
    ]
    try:
        res = run_bass_kernel_spmd(nc, in_maps, core_ids=list(range(N_CORES)))
    except ModuleNotFoundError:
        # BASS_TRACE set but the axon NTFF hook module isn't installed in
        # this container; retry with tracing force-disabled.
        os.environ["BASS_NEVER_TRACE"] = "1"
        res = run_bass_kernel_spmd(nc, in_maps, core_ids=list(range(N_CORES)))
    if res.exec_time_ns is not None:
        LAST_EXEC_NS = res.exec_time_ns

    out = np.empty((batch, N_FEAT), dtype=np.float32)
    for k in range(N_CORES):
        out[k * bpc:(k + 1) * bpc, :] = _unpack_out(res.results[k]["outqT"], chunk, bias_f)
    return out


def sim_time_ns(bpc=4096):
    """Deterministic single-core span from the instruction cost model
    (TimelineSim). All 8 cores run this same program in parallel."""
    from concourse.timeline_sim import TimelineSim

    key = ("v5", bpc, CHUNK)
    if key not in _CACHE:
        _CACHE[key] = _build_program_v5(bpc, chunk=CHUNK)
    return TimelineSim(_CACHE[key], trace=False).simulate()


# revision 44
# speedup vs baseline: 1.0048x; 1.0048x over previous
"""Butterfly (nn_Butterfly) kernel for 8 Trainium2 NeuronCores — v5.

Math: stages 0-4 of the 10-stage butterfly mix features within contiguous
32-blocks (factor A), stages 5-9 mix features with equal (p mod 32)
(factor B); out = x A^T B^T + bias. Each core runs its 4096-sample batch
shard through three PE passes per 1024-sample chunk:

  phase A   8 matmuls per 128-sample slab with the *data* slab as the
            stationary operand and the A-factor tile as the moving one, so
            the PSUM result comes out sample-major (the first transpose of
            the feature exchange is fused into the matmul for free)
  T2        64 [128,128] PE transposes flip the intermediate back to
            feature-major, grouped so each transposed block is exactly one
            phase-B contraction tile (the full pi exchange = A-fusion + T2)
  phase B   16 matmuls per chunk; bias is added on the host during unpack

The PE stream is software-pipelined two chunks deep
(T2(j-1)/B(j-2) interleaved at slab granularity, then A(j)) so the PE
never stalls on PSUM evacuations; evacuations are spread across
DVE/ACT/Pool so no engine exceeds the DMA period. The DMA device (serial
~360GB/s in the cost model) carries only the 8MB input + 8MB output +
0.5MB weights per core; the feature exchange never touches it.
"""

import os
import numpy as np
import ml_dtypes

import concourse.bass as bass
import concourse.bacc as bacc
import concourse.mybir as mybir
import concourse.tile as tile
from concourse.bass_utils import run_bass_kernel_spmd

N_FEAT = 1024
M_STAGES = 10
N_CORES = 8

BF16 = ml_dtypes.bfloat16

LAST_EXEC_NS = None  # set when BASS_KERNEL_TRACE=1
CHUNK = 512

_CACHE = {}


def _apply_stages(x, twiddle, blocks):
    """Apply butterfly stages `blocks` to x [b, 1024] (mirrors reference)."""
    n = N_FEAT
    for m in blocks:
        s = 1 << m
        t = twiddle[0, m].reshape(n // (2 * s), s, 2, 2)
        o = x.reshape(-1, n // (2 * s), 2, s)
        x = np.einsum("gsij,bgjs->bgis", t, o).reshape(-1, n)
    return x


def _phase_mats(twiddle):
    """Device weight layouts.

    wa [128, 8*128]: tile c holds the A-factor for input features
      [128c, 128c+128); column f = 16*r_hi + 4*a_lo + r_lo is output
      feature 32*(4c + a_lo) + 4*r_hi + r_lo. Used as the MOVING operand
      (data slab stationary), so phase-A PSUM is sample-major.
    wb [128, 8*128]: tile r_hi contracts the 128 intermediate features
      q = 4*a + r_lo (feature 32a + 4 r_hi + r_lo) into outputs
      g = 4*a' + r_lo (feature 32a' + 4 r_hi + r_lo).
    """
    tw = twiddle.astype(np.float64)
    eye = np.eye(N_FEAT)
    a_full = _apply_stages(eye, tw, range(5)).T      # [out_feat, in_feat]
    b_full = _apply_stages(eye, tw, range(5, 10)).T
    f = np.arange(128)
    r_hi, a_lo, r_lo = f >> 4, (f >> 2) & 3, f & 3
    k = np.arange(128)
    wa = np.zeros((128, 8 * 128))
    wb = np.zeros((128, 8 * 128))
    for c in range(8):
        p_out = 32 * (4 * c + a_lo) + 4 * r_hi + r_lo
        p_in = 128 * c + k
        wa[:, c * 128:(c + 1) * 128] = a_full[np.ix_(p_out, p_in)].T
    g = np.arange(128)
    for rh in range(8):
        feat = 32 * (g >> 2) + 4 * rh + (g & 3)
        wb[:, rh * 128:(rh + 1) * 128] = b_full[np.ix_(feat, feat)].T
    return wa.astype(BF16), wb.astype(BF16)


def _pack_xt(shard_bf, chunk):
    """[bpc, 1024] bf16 -> [128, 8*bpc] slab-major device layout:
    xt[k, j*8*chunk + s*1024 + c*128 + n0] = x[j*chunk + s*128 + n0, c*128 + k]."""
    bpc = shard_bf.shape[0]
    nch = bpc // chunk
    ns = chunk // 128
    a = shard_bf.reshape(nch, ns, 128, 8, 128)      # [j, s, n0, c, k]
    return np.ascontiguousarray(
        a.transpose(4, 0, 1, 3, 2).reshape(128, 8 * bpc)
    )


def _unpack_out(raw, chunk, bias):
    """device out [128, 8*bpc] -> [bpc, 1024] f32 (+ bias on host).
    raw[g, j*8*chunk + rh*chunk + n] = out[j*chunk + n, 32*(g>>2) + 4*rh + (g&3)].
    """
    bpc = raw.shape[1] // 8
    nch = bpc // chunk
    raw = np.asarray(raw).astype(np.float32)
    a = raw.reshape(32, 4, nch, 8, chunk)           # [a', r_lo, j, rh, n]
    out = np.ascontiguousarray(a.transpose(2, 4, 0, 3, 1).reshape(bpc, N_FEAT))
    out += bias[None, :]
    return out


def _build_program_v5(bpc, chunk=1024, eva=None, evb=None, xbar_every=0):
    """Three-PE-pass butterfly for one core's shard (see module docstring)."""
    from concourse.bass import AP as _AP
    from concourse.bass import _add_dep_helper

    assert chunk % 512 == 0 and bpc % chunk == 0
    nch = bpc // chunk
    F = 8 * chunk                      # columns per chunk in xin/M/saq/ot
    NS = chunk // 128                  # sample slabs per chunk

    nc = bacc.Bacc("TRN2", debug=False)
    xt_d = nc.dram_tensor("xt", [128, 8 * bpc], mybir.dt.bfloat16, kind="ExternalInput").ap()
    wa_d = nc.dram_tensor("wa", [128, 8 * 128], mybir.dt.bfloat16, kind="ExternalInput").ap()
    wb_d = nc.dram_tensor("wb", [128, 8 * 128], mybir.dt.bfloat16, kind="ExternalInput").ap()
    id_d = nc.dram_tensor("idm", [128, 128], mybir.dt.bfloat16, kind="ExternalInput").ap()
    out_d = nc.dram_tensor("outqT", [128, 8 * bpc], mybir.dt.bfloat16, kind="ExternalOutput").ap()

    with tile.TileContext(nc) as tc:
        with (
            tc.tile_pool(name="wpool", bufs=1) as w_pool,
            tc.tile_pool(name="xin", bufs=4) as xin_pool,
            tc.tile_pool(name="mtile", bufs=2) as m_pool,
            tc.tile_pool(name="saq", bufs=2) as saq_pool,
            tc.tile_pool(name="otile", bufs=4) as out_pool,
            tc.tile_pool(name="psa", bufs=3, space="PSUM") as psa_pool,
            tc.tile_pool(name="pst", bufs=2, space="PSUM") as pst_pool,
            tc.tile_pool(name="psb", bufs=3, space="PSUM") as psb_pool,
        ):
            wa_sb = w_pool.tile([128, 8 * 128], mybir.dt.bfloat16, name="wa_sb")
            wb_sb = w_pool.tile([128, 8 * 128], mybir.dt.bfloat16, name="wb_sb")
            id_sb = w_pool.tile([128, 128], mybir.dt.bfloat16, name="id_sb")

            # SP issues almost every DMA. The first two transfers (wa + the
            # first input quarter) go out on scalar's HWDGE concurrently so
            # the ~1.3us issue latency of the two queues overlaps.
            nc.scalar.dma_start(wa_sb[:], wa_d[:])

            xins = []

            def load_chunk(j, pieces=2, first_eng=None):
                xin = xin_pool.tile([128, F], mybir.dt.bfloat16, name="xin")
                w = F // pieces
                for h in range(pieces):
                    eng = first_eng if (h == 0 and first_eng is not None) else nc.sync
                    eng.dma_start(
                        xin[:, h * w:(h + 1) * w],
                        xt_d[:, j * F + h * w: j * F + (h + 1) * w],
                    )
                xins.append(xin)

            load_chunk(0, pieces=4)
            nc.sync.dma_start(wb_sb[:], wb_d[:])
            nc.sync.dma_start(id_sb[:], id_d[:])
            if nch > 1:
                load_chunk(1)
            if nch > 2:
                load_chunk(2)

            # engine split patterns. GPSIMD cannot access PSUM on TRN2
            # (BIR verifier), so every evacuation is on DVE or ACT:
            # per chunk DVE carries the 8 evacT2 (2x bf16) + 11 f32 halves,
            # ACT the other 21 halves (~12us each, the pipeline period).
            EMAP = {"S": nc.scalar, "V": nc.vector}
            EVA = [EMAP[ch] for ch in (eva or "VSSSVSSSVSSSVSSS")]     # evacA
            EVB = [EMAP[ch] for ch in (evb or "SVSVSVSSVVSSVSSV")]     # evacB
            EVB_TAIL = [EMAP[ch] for ch in "VSVSVSVSVSVSVSVS"]
            # tail chunks: ACT's backlog gates the drain; shift the last
            # chunks' evacA to DVE, which idles there.
            EVA_TAIL = [EMAP[ch] for ch in "VSVSVSVSVSVSVSVS"]

            def copy_on(eng, dst, src):
                if eng is nc.scalar:
                    return eng.activation(dst, src, mybir.ActivationFunctionType.Identity)
                return eng.tensor_copy(dst, src)

            # per-chunk state
            st = {}
            for j in range(nch):
                st[j] = {"evacA": {}, "evacT2": {}, "M": None, "saq": None,
                         "ot": None}

            def emit_A_slab(j, s):
                xin = xins[j]
                if st[j]["M"] is None:
                    st[j]["M"] = m_pool.tile([128, F], mybir.dt.bfloat16, name="mtile")
                    st[j]["evacA"] = {ss: [] for ss in range(NS)}
                m_h = st[j]["M"][:].tensor
                for h in range(2):
                    psa = psa_pool.tile([128, 512], mybir.dt.float32, name="psa")
                    for cc in range(4):
                        c = 4 * h + cc
                        nc.tensor.matmul(
                            psa[:, cc * 128:(cc + 1) * 128],
                            xin[:, s * 1024 + c * 128: s * 1024 + (c + 1) * 128],
                            wa_sb[:, c * 128:(c + 1) * 128],
                            start=True, stop=True,
                        )
                    # psa col cc*128 + (16 r_hi + z) -> M col
                    #   s*1024 + r_hi*128 + (4h+cc)*16 + z
                    dst = _AP(
                        m_h, s * 1024 + 64 * h,
                        [[F, 128], [16, 4], [128, 8], [1, 16]],
                    )
                    pat = EVA_TAIL if j >= nch - 1 else EVA
                    cp = copy_on(pat[(s * 2 + h) % len(pat)], dst, psa[:])
                    st[j]["evacA"][s].append(cp)

            def alloc_saq(j):
                saq = saq_pool.tile([128, F], mybir.dt.bfloat16, name="saq")
                st[j]["saq"] = saq
                st[j]["ot"] = out_pool.tile([128, F], mybir.dt.bfloat16, name="ot")

            def emit_T2_slab(j, s, use_xbar=False):
                M = st[j]["M"]
                saq = st[j]["saq"]
                if use_xbar:
                    # X-bar DMA transpose straight into saq; spends DMA-device
                    # slack to relieve the saturated DVE/PE.
                    last = None
                    for rh in range(8):
                        tp = nc.sync.dma_start_transpose(
                            saq[:, rh * chunk + s * 128: rh * chunk + (s + 1) * 128],
                            M[:, s * 1024 + rh * 128: s * 1024 + (rh + 1) * 128],
                        )
                        for cp in st[j]["evacA"][s]:
                            _add_dep_helper(tp.ins, cp.ins, sync=True,
                                            reason="xbar reads evacA raw-AP writes")
                        last = tp
                    st[j]["evacT2"][s] = last
                    return
                pst = pst_pool.tile([128, 1024], mybir.dt.bfloat16, name="pst")
                for rh in range(8):
                    tp = nc.tensor.transpose(
                        pst[:, rh * 128:(rh + 1) * 128],
                        M[:, s * 1024 + rh * 128: s * 1024 + (rh + 1) * 128],
                        id_sb[:],
                    )
                    for cp in st[j]["evacA"][s]:
                        _add_dep_helper(tp.ins, cp.ins, sync=True,
                                        reason="T2 reads evacA raw-AP writes")
                # pst col rh*128 + n0 -> saq col rh*chunk + s*128 + n0
                dst = _AP(saq[:].tensor, s * 128, [[F, 128], [chunk, 8], [1, 128]])
                st[j]["evacT2"][s] = nc.vector.tensor_copy(dst, pst[:])

            def emit_B_tile(j, rh):
                saq = st[j]["saq"]
                ot = st[j]["ot"]
                for t in range(chunk // 512):
                    psb = psb_pool.tile([128, 512], mybir.dt.float32, name="psb")
                    mm = nc.tensor.matmul(
                        psb[:],
                        wb_sb[:, rh * 128:(rh + 1) * 128],
                        saq[:, rh * chunk + t * 512: rh * chunk + (t + 1) * 512],
                        start=True, stop=True,
                    )
                    for s in range(4 * t, 4 * t + 4):
                        _add_dep_helper(mm.ins, st[j]["evacT2"][s].ins, sync=True,
                                        reason="B reads evacT2 raw-AP writes")
                    dst = ot[:, rh * chunk + t * 512: rh * chunk + (t + 1) * 512]
                    copy_on((EVB_TAIL if j >= nch - 1 else EVB)[(rh * (chunk // 512) + t) % len(EVB)], dst, psb[:])

            def emit_out(j, q, pieces=4):
                w = F // pieces
                nc.sync.dma_start(
                    out_d[:, j * F + q * w: j * F + (q + 1) * w],
                    st[j]["ot"][:, q * w:(q + 1) * w],
                )

            # software pipeline, two chunks deep on the PE stream, fully
            # interleaved at slab granularity so every PSUM slot has ~3.8us
            # of PE work between produce and reuse:
            #   step j, slab s: T2(j-1, s); B(j-2, rh=s) both halves; A(j, s)
            # Input loads go at step START (xin bufs=4 makes them dep-free
            # immediately) so the serial DMA device never sits behind an
            # output that is still waiting on its evacuation.
            RPB = 8 // NS
            for j in range(nch + 2):
                if j < nch and j + 3 < nch:
                    load_chunk(j + 3)
                if 1 <= j <= nch:
                    alloc_saq(j - 1)
                for s in range(NS):
                    if 1 <= j <= nch:
                        emit_T2_slab(j - 1, s)
                    if 2 <= j:
                        for rr in range(RPB):
                            emit_B_tile(j - 2, s * RPB + rr)
                        emit_out(j - 2, s, pieces=NS)
                    if j < nch:
                        emit_A_slab(j, s)

    nc.compile()
    return nc


def kernel(x, twiddle, bias):
    global LAST_EXEC_NS
    batch = x.shape[0]
    assert batch % N_CORES == 0
    bpc = batch // N_CORES
    chunk = CHUNK

    # ---- host prep ----
    wa, wb = _phase_mats(np.asarray(twiddle, dtype=np.float32))
    bias_f = np.asarray(bias, dtype=np.float32)
    idm = np.eye(128, dtype=BF16)
    x_bf = np.asarray(x).astype(BF16)
    shards = [
        _pack_xt(x_bf[k * bpc:(k + 1) * bpc, :], chunk)
        for k in range(N_CORES)
    ]

    key = ("v5", bpc, chunk)
    if key not in _CACHE:
        _CACHE[key] = _build_program_v5(bpc, chunk=chunk)
    nc = _CACHE[key]

    in_maps = [
        {"xt": shards[k], "wa": wa, "wb": wb, "idm": idm}
        for k in range(N_CORES)
    ]
    try:
        res = run_bass_kernel_spmd(nc, in_maps, core_ids=list(range(N_CORES)))
    except ModuleNotFoundError:
        # BASS_TRACE set but the axon NTFF hook module isn't installed in
        # this container; retry with tracing force-disabled.
        os.environ["BASS_NEVER_TRACE"] = "1"
        res = run_bass_kernel_spmd(nc, in_maps, core_ids=list(range(N_CORES)))
    if res.exec_time_ns is not None:
        LAST_EXEC_NS = res.exec_time_ns

    out = np.empty((batch, N_FEAT), dtype=np.float32)
    for k in range(N_CORES):
        out[k * bpc:(k + 1) * bpc, :] = _unpack_out(res.results[k]["outqT"], chunk, bias_f)
    return out


def sim_time_ns(bpc=4096):
    """Deterministic single-core span from the instruction cost model
    (TimelineSim). All 8 cores run this same program in parallel."""
    from concourse.timeline_sim import TimelineSim

    key = ("v5", bpc, CHUNK)
    if key not in _CACHE:
        _CACHE[key] = _build_program_v5(bpc, chunk=CHUNK)
    return TimelineSim(_CACHE[key], trace=False).simulate()
